# revision 1
# baseline (speedup 1.0000x reference)
"""Causal self-attention Trainium2 kernel (Bass/Tile), 8 NeuronCores.

Problem: B=2, S=2048, D=1024, H=16 heads (hd=64), fp32.
    qkv = x @ qkv_w + qkv_b ; per-head causal attention ; y = out @ out_w + out_b

Sharding (hybrid data x tensor parallel):
    8 cores = 2 batch groups x 4 head groups. Core c handles batch c//4 and
    the 4 heads [4*(c%4) .. 4*(c%4)+3]. Each core computes its partial
    out-projection y_c [S, D]; host sums the 4 partials per batch + out_b.

Per-core layout strategy (everything contraction-friendly, zero on-chip
transposes):
    - host supplies xT = x[b].T [D, S] so D is the DMA partition dim
    - qkv^T is computed directly: qkT [hd_n on partitions, S free]
    - scores are computed transposed: sT[k, q] = kT.T @ qT, softmax uses no
      max-subtraction (scores are O(6) so exp is safe in fp32), the softmax
      denominator comes out of the PV matmul via a ones-column appended to V,
      and the normalization divides after PV.
    - out^T accumulates in [hd_local=256 partitions, S] layout, which is
      exactly the lhsT the out-projection needs.
Matmuls run as float32r (full-rate fp32 path on TRN2 PE for free dim >= 256).
"""

import os
import sys

for _p in ("/opt/trn_rl_repo", "/root/.axon_site/_ro/trn_rl_repo"):
    if os.path.isdir(_p) and _p not in sys.path:
        sys.path.insert(0, _p)

import numpy as np
from contextlib import ExitStack

import concourse.bass as bass
import concourse.tile as tile
from concourse import bacc, mybir
from concourse.bass_utils import run_bass_kernel_spmd

B, S, D = 2, 2048, 1024
H, HD = 16, 64
NCORES = 8
LOCAL_H = 4           # heads per core
P = 128
KO = D // P           # 8 contraction sub-tiles for the projections
NQ = S // 512         # 4 q-tiles of 512
NKT = S // P          # 16 k-blocks of 128
F32 = mybir.dt.float32
F32R = mybir.dt.float32r
AF = mybir.ActivationFunctionType
SCALE = 1.0 / np.sqrt(HD)


def _emit(tc, nc, xT, wqk, wv, wo, bqkv, b65, onesd, y, has_qkv_bias):
    with ExitStack() as ctx:
        consts = ctx.enter_context(tc.tile_pool(name="consts", bufs=1))
        persis = ctx.enter_context(tc.tile_pool(name="persist", bufs=1))
        psum = ctx.enter_context(tc.tile_pool(name="ps", bufs=2, space="PSUM"))
        psum_o = ctx.enter_context(tc.tile_pool(name="pso", bufs=2, space="PSUM"))
        xstack = ctx.enter_context(ExitStack())
        xpool = xstack.enter_context(tc.tile_pool(name="xp", bufs=KO))

        # ---- constant loads (wqk/x interleaved per-ko so qkT starts early) ----
        b65_sb = consts.tile([1, 260], F32R)
        nc.scalar.dma_start(b65_sb[:], b65[None, :])
        ones_col = consts.tile([1, P], F32R)
        nc.scalar.dma_start(ones_col[:], onesd[None, :])
        # lower-triangle keep-mask for diagonal 128x128 score blocks
        mask128 = consts.tile([P, P], F32R)
        nc.scalar.dma_start(mask128[:], onesd[None, :].to_broadcast((P, P)))
        nc.gpsimd.affine_select(
            out=mask128[:], in_=mask128[:], pattern=[[1, P]],
            compare_op=mybir.AluOpType.is_ge, fill=0.0, base=0,
            channel_multiplier=-1,
        )
        if has_qkv_bias:
            bqk_sb = consts.tile([P, 4], F32)
            nc.scalar.dma_start(bqk_sb[:], bqkv[0:512].rearrange("(m p) -> p m", p=P))

        x_sb, wqk_t, wv_t = [], [], []
        for ko in range(KO):
            w = consts.tile([P, 512], F32R, name=f"wqk{ko}")
            nc.sync.dma_start(w[:], wqk[ko * P:(ko + 1) * P, :])
            wqk_t.append(w)
            t = xpool.tile([P, S], F32R, tag="x")
            nc.sync.dma_start(t[:], xT[ko * P:(ko + 1) * P, :])
            x_sb.append(t)
        for ko in range(KO):
            w = consts.tile([P, 260], F32R, name=f"wv{ko}")
            nc.sync.dma_start(w[:], wv[ko * P:(ko + 1) * P, :])
            wv_t.append(w)
        wo_sb = consts.tile([P, 2, D], F32R)
        nc.sync.dma_start(wo_sb[:], wo.rearrange("(ks p) n -> p ks n", p=P))

        # persistent activations
        qkT = persis.tile([P, 4, S], F32R)       # m-tiles 0,1: qT(h0..h3); 2,3: kT
        v_all = persis.tile([P, NKT, LOCAL_H, 65], F32R)  # [k-part, kt, lh, hd|ones]
        outT = persis.tile([P, 2, S], F32R)      # attention out^T (out-proj lhsT)

        # ---- qk^T projection: qkT[m] = (wqk[:, m-slice]).T @ xT ----
        for m in range(4):
            for n in range(NQ):
                gidx = m * NQ + n
                pool_ = psum if gidx % 2 == 0 else psum_o
                ps = pool_.tile([P, 512], F32, tag="mm512" if gidx % 2 == 0 else "o",
                                name=f"qk{gidx}")
                for ko in range(KO):
                    nc.tensor.matmul(
                        ps[:],
                        (wqk_t[ko][:, m * P:(m + 1) * P]),
                        (x_sb[ko][:, n * 512:(n + 1) * 512]),
                        start=(ko == 0), stop=(ko == KO - 1),
                    )
                dst = qkT[:, m, n * 512:(n + 1) * 512]
                if has_qkv_bias:
                    nc.scalar.activation(dst, ps[:], AF.Identity, bias=bqk_sb[:, m:m + 1])
                else:
                    nc.vector.tensor_copy(dst, ps[:])

        # ---- v projection (natural layout, ones/bias col via K=1 matmul) ----
        for mt in range(NKT):
            pool_ = psum if mt % 2 == 0 else psum_o
            ps = pool_.tile([P, 512], F32, tag="mm512" if mt % 2 == 0 else "o",
                            name=f"vp{mt}")
            pv = ps[:, 0:260]
            for ko in range(KO):
                nc.tensor.matmul(
                    pv,
                    (x_sb[ko][:, mt * P:(mt + 1) * P]),
                    (wv_t[ko][:]),
                    start=(ko == 0), stop=False,
                )
            nc.tensor.matmul(pv, (ones_col[:1, :]), (b65_sb[:1, :]),
                             start=False, stop=True)
            nc.vector.tensor_copy(
                v_all[:, mt, :, :],
                pv.rearrange("p (h d) -> p h d", h=LOCAL_H),
            )

        # x tiles are dead now; release their SBUF for the attention pools
        xstack.close()
        work = ctx.enter_context(tc.tile_pool(name="work", bufs=4))
        small = ctx.enter_context(tc.tile_pool(name="small", bufs=3))

        # ---- attention (jq outer so out-proj can stream per q-tile) ----
        for jq in range(NQ):
            for hp in range(2):        # local heads (2hp, 2hp+1)
                po = [psum_o.tile([65, 512], F32, tag="o", name=f"po{i_}")
                      for i_ in range(2)]
                last_kt = 4 * jq + 3
                for kt in range(last_kt + 1):
                    # diagonal blocks: columns q < 128*rel are fully masked;
                    # compute only [f0, 512) and mask just the 128-wide triangle
                    rel = kt - 4 * jq
                    f0 = 128 * rel if rel > 0 else 0
                    ps = psum.tile([P, 2, 512], F32, tag="s")
                    for i in range(2):
                        poff = 64 * i
                        nc.tensor.matmul(
                            ps[:, i, f0:512],
                            (qkT[poff:poff + 64, 2 + hp, kt * P:(kt + 1) * P]),
                            (qkT[poff:poff + 64, hp,
                                 jq * 512 + f0:(jq + 1) * 512]),
                            start=True, stop=True,
                        )
                    et = work.tile([P, 2, 512], F32R, tag="e")
                    nc.scalar.activation(et[:, :, f0:512], ps[:, :, f0:512],
                                         AF.Exp, scale=float(SCALE))
                    if rel >= 0:   # mask the 128-wide triangle at [f0, f0+128)
                        nc.vector.tensor_tensor(
                            et[:, 0, f0:f0 + 128], et[:, 0, f0:f0 + 128],
                            mask128[:], mybir.AluOpType.mult)
                        nc.vector.tensor_tensor(
                            et[:, 1, f0:f0 + 128], et[:, 1, f0:f0 + 128],
                            mask128[:], mybir.AluOpType.mult)
                    for i in range(2):
                        lh = 2 * hp + i
                        nc.tensor.matmul(
                            po[i][:, f0:512],
                            (v_all[:, kt, lh, :]),
                            (et[:, i, f0:512]),
                            start=(kt == 0), stop=(kt == last_kt),
                        )
                # stage po out of PSUM immediately (frees the bank for the
                # next head-pair), then normalize off-PSUM.
                # 1/l split across engines: i=0 DVE reciprocal, i=1 ACT
                # exp(-ln(l)) (Ln/Exp share the loaded table set).
                for i in range(2):
                    st = work.tile([65, 512], F32, tag="st")
                    nc.vector.tensor_copy(st[:], po[i][:])
                    rr = small.tile([1, 512], F32R, tag="rr")
                    if i == 0:
                        rf = small.tile([1, 512], F32, tag="rf")
                        nc.vector.reciprocal(rf[:], st[64:65, :])
                        nc.vector.tensor_copy(rr[:], rf[:])
                    else:
                        lr = small.tile([1, 512], F32, tag="lr")
                        nc.scalar.activation(lr[:], st[64:65, :], AF.Ln)
                        nc.scalar.activation(rr[:], lr[:], AF.Exp, scale=-1.0)
                    rb_ps = psum_o.tile([64, 512], F32, tag="o", name="rbps")
                    nc.tensor.matmul(rb_ps[:], ones_col[:1, 0:64], rr[:1, :],
                                     start=True, stop=True)
                    nc.vector.tensor_tensor(
                        outT[64 * i:64 * i + 64, hp, jq * 512:(jq + 1) * 512],
                        st[0:64, :], rb_ps[:], mybir.AluOpType.mult,
                    )
            # ---- out-projection for this q-tile's 4 seq sub-tiles ----
            for mt in range(4 * jq, 4 * jq + 4):
                for n2 in range(2):
                    ps = psum.tile([P, 512], F32, tag="mm512")
                    for ks in range(2):
                        nc.tensor.matmul(
                            ps[:],
                            (outT[:, ks, mt * P:(mt + 1) * P]),
                            (wo_sb[:, ks, n2 * 512:(n2 + 1) * 512]),
                            start=(ks == 0), stop=(ks == 1),
                        )
                    yt = work.tile([P, 512], F32, tag="y")
                    nc.vector.tensor_copy(yt[:], ps[:])
                    nc.gpsimd.dma_start(
                        y[mt * P:(mt + 1) * P, n2 * 512:(n2 + 1) * 512], yt[:])


def build_nc(has_qkv_bias):
    nc = bacc.Bacc("TRN2", target_bir_lowering=False, debug=False,
                   num_devices=NCORES)
    xT = nc.dram_tensor("xT", [D, S], F32R, kind="ExternalInput")
    wqk = nc.dram_tensor("wqk", [D, 512], F32R, kind="ExternalInput")
    wv = nc.dram_tensor("wv", [D, 260], F32R, kind="ExternalInput")
    wo = nc.dram_tensor("wo", [2 * P, D], F32R, kind="ExternalInput")
    bqkv = nc.dram_tensor("bqkv", [768], F32, kind="ExternalInput")
    b65 = nc.dram_tensor("b65", [260], F32R, kind="ExternalInput")
    onesd = nc.dram_tensor("onesd", [P], F32R, kind="ExternalInput")
    y = nc.dram_tensor("y", [S, D], F32, kind="ExternalOutput")
    with tile.TileContext(nc) as tc:
        _emit(tc, nc, xT.ap(), wqk.ap(), wv.ap(), wo.ap(), bqkv.ap(), b65.ap(),
              onesd.ap(), y.ap(), has_qkv_bias)
    nc.compile()
    return nc


_NC_CACHE = {}


def _get_nc(has_qkv_bias):
    key = bool(has_qkv_bias)
    if key not in _NC_CACHE:
        _NC_CACHE[key] = build_nc(key)
    return _NC_CACHE[key]


def _round_fp32r(a):
    """Round fp32 to the fp32r grid (11-bit mantissa; low 12 bits zero, RNE)."""
    u = np.ascontiguousarray(a, dtype=np.float32).view(np.uint32)
    u = (u + 0x7FF + ((u >> 12) & 1)) & np.uint32(0xFFFFF000)
    return u.view(np.float32)


def make_in_maps(x, qkv_w, qkv_b, out_w):
    """Per-core host-side sharding. Core c: batch c//4, heads 4*(c%4)..+3."""
    in_maps = []
    xTs = [_round_fp32r(np.ascontiguousarray(x[b].T)) for b in range(B)]
    for c in range(NCORES):
        b = c // (NCORES // B)
        g = c % (NCORES // B)
        h0 = LOCAL_H * g
        cols = slice(h0 * HD, (h0 + LOCAL_H) * HD)
        wq = qkv_w[:, cols]
        wk = qkv_w[:, D:][:, cols]
        wv_ = qkv_w[:, 2 * D:][:, cols]
        bq = qkv_b[cols]
        bk = qkv_b[D:][cols]
        bv = qkv_b[2 * D:][cols]
        wv_pad = np.zeros((D, LOCAL_H, 65), np.float32)
        wv_pad[:, :, :64] = wv_.reshape(D, LOCAL_H, HD)
        b65_arr = np.zeros((LOCAL_H, 65), np.float32)
        b65_arr[:, :64] = bv.reshape(LOCAL_H, HD)
        b65_arr[:, 64] = 1.0
        in_maps.append({
            "xT": xTs[b],
            "wqk": _round_fp32r(np.concatenate([wq, wk], axis=1)),
            "wv": _round_fp32r(wv_pad.reshape(D, LOCAL_H * 65)),
            "wo": _round_fp32r(out_w[cols, :]),
            "bqkv": np.ascontiguousarray(np.concatenate([bq, bk, bv])),
            "b65": _round_fp32r(b65_arr.reshape(-1)),
            "onesd": np.ones(P, np.float32),
        })
    return in_maps


def _ensure_ntff_hook():
    """Provide antenv.axon_hooks (missing in this image) so trace=True works."""
    try:
        from antenv.axon_hooks import get_axon_ntff_profile_hook  # noqa: F401
        return
    except ImportError:
        pass
    import types
    import antenv
    mod = types.ModuleType("antenv.axon_hooks")
    holder = {"hook": None}
    mod.set_axon_ntff_profile_hook = lambda h: holder.__setitem__("hook", h)
    mod.get_axon_ntff_profile_hook = lambda: holder["hook"]
    sys.modules["antenv.axon_hooks"] = mod
    antenv.axon_hooks = mod
    try:
        from trn_agent_boot.trn_boot import _ntff_profile_via_ctypes
        so = "/opt/axon/libaxon_pjrt.so"
        if os.path.exists(so):
            mod.set_axon_ntff_profile_hook(_ntff_profile_via_ctypes(so))
    except Exception:
        pass


def kernel(x, qkv_w, qkv_b, out_w, out_b, _trace=False):
    if _trace:
        _ensure_ntff_hook()
    x = np.asarray(x, dtype=np.float32)
    qkv_w = np.asarray(qkv_w, dtype=np.float32)
    qkv_b = np.asarray(qkv_b, dtype=np.float32)
    out_w = np.asarray(out_w, dtype=np.float32)
    out_b = np.asarray(out_b, dtype=np.float32)

    has_qkv_bias = bool(np.any(qkv_b))
    nc = _get_nc(has_qkv_bias)
    in_maps = make_in_maps(x, qkv_w, qkv_b, out_w)
    res = run_bass_kernel_spmd(nc, in_maps, core_ids=list(range(NCORES)),
                               trace=_trace)
    y = np.zeros((B, S, D), dtype=np.float32)
    for c in range(NCORES):
        y[c // (NCORES // B)] += res.results[c]["y"]
    y += out_b
    if _trace:
        kernel.last_results = res
    return y



# revision 8
# speedup vs baseline: 1.4424x; 1.4424x over previous
"""Causal self-attention Trainium2 kernel (Bass/Tile), 8 NeuronCores.

Problem: B=2, S=2048, D=1024, H=16 heads (hd=64), fp32.
    qkv = x @ qkv_w + qkv_b ; per-head causal attention ; y = out @ out_w + out_b

Sharding (hybrid data x tensor parallel):
    8 cores = 2 batch groups x 4 head groups. Core c handles batch c//4 and
    the 4 heads [4*(c%4) .. 4*(c%4)+3]. Each core computes its partial
    out-projection y_c [S, D]; host sums the 4 partials per batch + out_b.

Per-core design (v2 — bf16 datapath):
    - all matmul operands bf16 (1 cycle/row on the PE vs ~2 for fp32r, and
      FastWeightLoad halves LDWEIGHTS); PSUM accumulation stays fp32.
    - scores computed transposed sT[k, q] with the two heads of an m-tile
      row-packed into the 128-row array (tile_position via base partitions).
    - softmax denominator comes out of the PV matmul via a ones-column
      appended to V (planted once by a memset); normalization uses
      reciprocal_approx_fast + a tiny broadcast matmul, then one DVE
      multiply into the bf16 out^T staging tile.
    - the qkv / out projections are EMITTED INTERLEAVED with the attention
      blocks so the PE processes projection matmuls while the scalar engine
      (the attention bottleneck: exp) works through score tiles.
"""

import os
import sys

for _p in ("/opt/trn_rl_repo", "/root/.axon_site/_ro/trn_rl_repo"):
    if os.path.isdir(_p) and _p not in sys.path:
        sys.path.insert(0, _p)

import numpy as np
import ml_dtypes
from contextlib import ExitStack

import concourse.bass as bass
import concourse.tile as tile
from concourse import bacc, mybir
from concourse.bass_utils import run_bass_kernel_spmd

B, S, D = 2, 2048, 1024
H, HD = 16, 64
NCORES = 8
LOCAL_H = 4           # heads per core
P = 128
KO = D // P           # 8 contraction sub-tiles for the projections
NQ = S // 512         # 4 q-tiles of 512
NKT = S // P          # 16 k-blocks of 128
F32 = mybir.dt.float32
F32R = mybir.dt.float32r
BF16 = mybir.dt.bfloat16
AF = mybir.ActivationFunctionType
SCALE = 1.0 / np.sqrt(HD)
BF = ml_dtypes.bfloat16


def _emit(tc, nc, xT, wqk, wv, wo, bqk, bv, maskd, ones64d, ones128d, y,
          has_qkv_bias):
    with ExitStack() as ctx:
        consts = ctx.enter_context(tc.tile_pool(name="consts", bufs=1))
        persis = ctx.enter_context(tc.tile_pool(name="persist", bufs=1))
        # PSUM: pp 2x1 bank (proj + rb), ps 2x2 banks (scores),
        # po 1x2 banks (PV accum pair) -> 8 banks total
        pp = ctx.enter_context(tc.tile_pool(name="pp", bufs=2, space="PSUM"))
        ps = ctx.enter_context(tc.tile_pool(name="ps", bufs=2, space="PSUM"))
        pop = ctx.enter_context(tc.tile_pool(name="po", bufs=1, space="PSUM"))
        work = ctx.enter_context(tc.tile_pool(name="work", bufs=4))
        ypool = ctx.enter_context(tc.tile_pool(name="yp", bufs=3))
        small = ctx.enter_context(tc.tile_pool(name="small", bufs=2))

        # ---- constants ----
        mask128 = consts.tile([P, P], BF16)
        nc.gpsimd.dma_start(mask128[:], maskd[:, :])
        ones64b_sb = consts.tile([1, 64], BF16)
        nc.gpsimd.dma_start(ones64b_sb[:], ones64d[None, :])
        if has_qkv_bias:
            bqk_sb = consts.tile([P, 4], F32)
            nc.gpsimd.dma_start(bqk_sb[:], bqk.rearrange("(m p) -> p m", p=P))
            bv_sb = consts.tile([1, 256], BF16)
            nc.gpsimd.dma_start(bv_sb[:], bv[None, :])
            ones128_sb = consts.tile([1, P], BF16)
            nc.gpsimd.dma_start(ones128_sb[:], ones128d[None, :])

        # ---- weight / activation input DMAs (fine-grained so the first
        #      projection group can start ~4us in) ----
        wqk_t, wv_t = [], []
        x_t = [[None] * NQ for _ in range(KO)]
        for ko in range(KO):
            w = consts.tile([P, 512], BF16, name=f"wqk{ko}")
            nc.gpsimd.dma_start(w[:], wqk[ko * P:(ko + 1) * P, :])
            wqk_t.append(w)
            t = persis.tile([P, 512], BF16, name=f"x{ko}_0")
            nc.sync.dma_start(t[:], xT[ko * P:(ko + 1) * P, 0:512])
            x_t[ko][0] = t
        for ko in range(KO):
            w = consts.tile([P, 256], BF16, name=f"wv{ko}")
            nc.gpsimd.dma_start(w[:], wv[ko * P:(ko + 1) * P, :])
            wv_t.append(w)
            t = persis.tile([P, 512], BF16, name=f"x{ko}_1")
            nc.sync.dma_start(t[:], xT[ko * P:(ko + 1) * P, 512:1024])
            x_t[ko][1] = t
        wo_sb = consts.tile([P, 2, D], BF16)
        nc.gpsimd.dma_start(wo_sb[:], wo.rearrange("(ks p) n -> p ks n", p=P))
        for n in (2, 3):
            for ko in range(KO):
                t = persis.tile([P, 512], BF16, name=f"x{ko}_{n}")
                nc.sync.dma_start(t[:], xT[ko * P:(ko + 1) * P,
                                            n * 512:(n + 1) * 512])
                x_t[ko][n] = t

        # persistent activations
        qkT = persis.tile([P, 4, S], BF16)        # m 0,1: qT(h0..h3); 2,3: kT
        v_all = persis.tile([P, NKT, LOCAL_H, 65], BF16)
        outT = persis.tile([P, 2, S], BF16)       # attention out^T (bf16)

        # plant the softmax-denominator ones column of V once
        nc.vector.memset(v_all[:, :, :, 64:65], 1.0)

        def qk_group(m, n):
            """qkT[m][n-slice] = (wqk[:, m*128:+128]).T @ xT[:, n*512:+512]"""
            t = pp.tile([P, 512], F32, tag="p", name=f"qk{m}_{n}")
            for ko in range(KO):
                nc.tensor.matmul(
                    t[:],
                    wqk_t[ko][:, m * P:(m + 1) * P],
                    x_t[ko][n][:],
                    start=(ko == 0), stop=(ko == KO - 1),
                )
            dst = qkT[:, m, n * 512:(n + 1) * 512]
            if has_qkv_bias:
                nc.scalar.activation(dst, t[:], AF.Identity,
                                     bias=bqk_sb[:, m:m + 1])
            else:
                nc.any.tensor_copy(dst, t[:])

        def v_group(mt):
            """v_all[:, mt] = x[mt-block] @ wv  (natural layout)"""
            t = pp.tile([P, 512], F32, tag="p", name=f"vp{mt}")
            pv = t[:, 0:256]
            last = KO - 1 if not has_qkv_bias else None
            for ko in range(KO):
                nc.tensor.matmul(
                    pv,
                    x_t[ko][mt // 4][:, (mt % 4) * P:(mt % 4 + 1) * P],
                    wv_t[ko][:],
                    start=(ko == 0),
                    stop=(ko == KO - 1 and not has_qkv_bias),
                )
            if has_qkv_bias:
                nc.tensor.matmul(pv, ones128_sb[:1, :], bv_sb[:1, :],
                                 start=False, stop=True)
            nc.any.tensor_copy(
                v_all[:, mt, :, 0:64],
                pv.rearrange("p (h d) -> p h d", h=LOCAL_H),
            )

        def attn_block(jq, hp):
            po_t = pop.tile([65, 2, 512], F32, tag="po", name=f"po{jq}_{hp}")
            last_kt = 4 * jq + 3
            for kt in range(last_kt + 1):
                rel = kt - 4 * jq
                f0 = 128 * rel if rel > 0 else 0
                s_t = ps.tile([P, 2, 512], F32, tag="s")
                for i in range(2):
                    poff = 64 * i
                    nc.tensor.matmul(
                        s_t[:, i, f0:512],
                        qkT[poff:poff + 64, 2 + hp, kt * P:(kt + 1) * P],
                        qkT[poff:poff + 64, hp,
                            jq * 512 + f0:(jq + 1) * 512],
                        start=True, stop=True,
                    )
                et = work.tile([P, 2, 512], BF16, tag="et")
                nc.scalar.activation(et[:, :, f0:512], s_t[:, :, f0:512],
                                     AF.Exp, scale=float(SCALE))
                if rel >= 0:   # mask the 128-wide triangle at [f0, f0+128)
                    for i in range(2):
                        nc.vector.tensor_tensor(
                            et[:, i, f0:f0 + 128], et[:, i, f0:f0 + 128],
                            mask128[:], mybir.AluOpType.mult)
                for i in range(2):
                    lh = 2 * hp + i
                    nc.tensor.matmul(
                        po_t[:, i, f0:512],
                        v_all[:, kt, lh, :],
                        et[:, i, f0:512],
                        start=(kt == 0), stop=(kt == last_kt),
                    )
            # normalize: stage po to SBUF (frees the PSUM pair early), 1/den
            # via fast DVE reciprocal, broadcast via a tiny matmul, one DVE
            # multiply into bf16 outT
            st = small.tile([64, 2, 512], F32, tag="st")
            nc.any.tensor_copy(st[:], po_t[0:64, :, :])
            den_t = small.tile([1, 2, 512], F32, tag="den")
            nc.any.tensor_copy(den_t[:], po_t[64:65, :, :])
            # NOTE: reciprocal_approx_fast requires its input at partition 0
            rf = small.tile([1, 2, 512], F32, tag="rf")
            nc.vector.reciprocal_approx_fast(rf[:], den_t[:])
            rr = small.tile([1, 2, 512], BF16, tag="rr")
            nc.any.tensor_copy(rr[:], rf[:])
            for i in range(2):
                rb = pp.tile([P, 512], F32, tag="p", name=f"rb{jq}_{hp}_{i}")
                nc.tensor.matmul(rb[0:64, :], ones64b_sb[:1, :],
                                 rr[:, i, :],
                                 start=True, stop=True)
                nc.vector.tensor_tensor(
                    outT[64 * i:64 * i + 64, hp, jq * 512:(jq + 1) * 512],
                    st[0:64, i, :], rb[0:64, :], mybir.AluOpType.mult)

        def out_proj(jq):
            for mt in range(4 * jq, 4 * jq + 4):
                for n2 in range(2):
                    t = pp.tile([P, 512], F32, tag="p", name=f"op{mt}_{n2}")
                    for ks in range(2):
                        nc.tensor.matmul(
                            t[:],
                            outT[:, ks, mt * P:(mt + 1) * P],
                            wo_sb[:, ks, n2 * 512:(n2 + 1) * 512],
                            start=(ks == 0), stop=(ks == 1),
                        )
                    yt = ypool.tile([P, 512], F32, tag="y")
                    nc.any.tensor_copy(yt[:], t[:])
                    nc.gpsimd.dma_start(
                        y[mt * P:(mt + 1) * P, n2 * 512:(n2 + 1) * 512],
                        yt[:])

        # ---- interleaved emission schedule ----
        # proj groups are emitted one attention block ahead of their use so
        # the PE always has projection work to fill exp-latency stalls.
        qk_group(2, 0); qk_group(0, 0)
        v_group(0); v_group(1); v_group(2); v_group(3)
        qk_group(3, 0); qk_group(1, 0)
        attn_block(0, 0)
        qk_group(2, 1); qk_group(0, 1)
        v_group(4); v_group(5); v_group(6); v_group(7)
        attn_block(0, 1)
        qk_group(3, 1); qk_group(1, 1)
        attn_block(1, 0)
        qk_group(2, 2); qk_group(0, 2)
        v_group(8); v_group(9); v_group(10); v_group(11)
        out_proj(0)
        attn_block(1, 1)
        qk_group(3, 2); qk_group(1, 2)
        attn_block(2, 0)
        qk_group(2, 3); qk_group(0, 3)
        v_group(12); v_group(13); v_group(14); v_group(15)
        out_proj(1)
        attn_block(2, 1)
        qk_group(3, 3); qk_group(1, 3)
        attn_block(3, 0)
        out_proj(2)
        attn_block(3, 1)
        out_proj(3)


def build_nc(has_qkv_bias):
    nc = bacc.Bacc("TRN2", target_bir_lowering=False, debug=False,
                   num_devices=NCORES)
    xT = nc.dram_tensor("xT", [D, S], BF16, kind="ExternalInput")
    wqk = nc.dram_tensor("wqk", [D, 512], BF16, kind="ExternalInput")
    wv = nc.dram_tensor("wv", [D, 256], BF16, kind="ExternalInput")
    wo = nc.dram_tensor("wo", [2 * P, D], BF16, kind="ExternalInput")
    bqk = nc.dram_tensor("bqk", [512], F32, kind="ExternalInput")
    bv = nc.dram_tensor("bv", [256], BF16, kind="ExternalInput")
    maskd = nc.dram_tensor("maskd", [P, P], BF16, kind="ExternalInput")
    ones64d = nc.dram_tensor("ones64d", [64], BF16, kind="ExternalInput")
    ones128d = nc.dram_tensor("ones128d", [P], BF16, kind="ExternalInput")
    y = nc.dram_tensor("y", [S, D], F32, kind="ExternalOutput")
    with tile.TileContext(nc) as tc:
        _emit(tc, nc, xT.ap(), wqk.ap(), wv.ap(), wo.ap(), bqk.ap(), bv.ap(),
              maskd.ap(), ones64d.ap(), ones128d.ap(), y.ap(), has_qkv_bias)
    nc.compile()
    return nc


_NC_CACHE = {}


def _get_nc(has_qkv_bias):
    key = bool(has_qkv_bias)
    if key not in _NC_CACHE:
        _NC_CACHE[key] = build_nc(key)
    return _NC_CACHE[key]


def make_in_maps(x, qkv_w, qkv_b, out_w):
    """Per-core host-side sharding. Core c: batch c//4, heads 4*(c%4)..+3."""
    in_maps = []
    xTs = [np.ascontiguousarray(x[b].T).astype(BF) for b in range(B)]
    # scores are stored transposed sT[k, q]: keep q >= k (upper triangle)
    mask = np.triu(np.ones((P, P), np.float32)).astype(BF)
    ones64 = np.ones(64, np.float32)
    ones128 = np.ones(P, np.float32).astype(BF)
    for c in range(NCORES):
        b = c // (NCORES // B)
        g = c % (NCORES // B)
        h0 = LOCAL_H * g
        cols = slice(h0 * HD, (h0 + LOCAL_H) * HD)
        wq = qkv_w[:, cols]
        wk = qkv_w[:, D:][:, cols]
        wv_ = qkv_w[:, 2 * D:][:, cols]
        bq = qkv_b[cols]
        bk = qkv_b[D:][cols]
        bv_ = qkv_b[2 * D:][cols]
        in_maps.append({
            "xT": xTs[b],
            "wqk": np.concatenate([wq, wk], axis=1).astype(BF),
            "wv": np.ascontiguousarray(wv_).astype(BF),
            "wo": np.ascontiguousarray(out_w[cols, :]).astype(BF),
            "bqk": np.ascontiguousarray(np.concatenate([bq, bk])),
            "bv": bv_.astype(BF),
            "maskd": mask,
            "ones64d": ones64.astype(BF),
            "ones128d": ones128,
        })
    return in_maps


def _ensure_ntff_hook():
    """Provide antenv.axon_hooks (missing in this image) so trace=True works."""
    try:
        from antenv.axon_hooks import get_axon_ntff_profile_hook  # noqa: F401
        return
    except ImportError:
        pass
    import types
    import antenv
    mod = types.ModuleType("antenv.axon_hooks")
    holder = {"hook": None}
    mod.set_axon_ntff_profile_hook = lambda h: holder.__setitem__("hook", h)
    mod.get_axon_ntff_profile_hook = lambda: holder["hook"]
    sys.modules["antenv.axon_hooks"] = mod
    antenv.axon_hooks = mod
    try:
        from trn_agent_boot.trn_boot import _ntff_profile_via_ctypes
        so = "/opt/axon/libaxon_pjrt.so"
        if os.path.exists(so):
            mod.set_axon_ntff_profile_hook(_ntff_profile_via_ctypes(so))
    except Exception:
        pass


def kernel(x, qkv_w, qkv_b, out_w, out_b, _trace=False):
    if _trace:
        _ensure_ntff_hook()
    x = np.asarray(x, dtype=np.float32)
    qkv_w = np.asarray(qkv_w, dtype=np.float32)
    qkv_b = np.asarray(qkv_b, dtype=np.float32)
    out_w = np.asarray(out_w, dtype=np.float32)
    out_b = np.asarray(out_b, dtype=np.float32)

    has_qkv_bias = bool(np.any(qkv_b))
    nc = _get_nc(has_qkv_bias)
    in_maps = make_in_maps(x, qkv_w, qkv_b, out_w)
    res = run_bass_kernel_spmd(nc, in_maps, core_ids=list(range(NCORES)),
                               trace=_trace)
    y = np.zeros((B, S, D), dtype=np.float32)
    for c in range(NCORES):
        y[c // (NCORES // B)] += res.results[c]["y"]
    y += out_b
    if _trace:
        kernel.last_results = res
    return y


# revision 13
# speedup vs baseline: 1.6957x; 1.1756x over previous
"""Causal self-attention Trainium2 kernel (Bass/Tile), 8 NeuronCores.

Problem: B=2, S=2048, D=1024, H=16 heads (hd=64), fp32.
    qkv = x @ qkv_w + qkv_b ; per-head causal attention ; y = out @ out_w + out_b

Sharding (hybrid data x tensor parallel):
    8 cores = 2 batch groups x 4 head groups. Core c handles batch c//4 and
    the 4 heads [4*(c%4) .. 4*(c%4)+3]. Each core computes its partial
    out-projection y_c [S, D]; host sums the 4 partials per batch + out_b.

Per-core design (v2 — bf16 datapath):
    - all matmul operands bf16 (1 cycle/row on the PE vs ~2 for fp32r, and
      FastWeightLoad halves LDWEIGHTS); PSUM accumulation stays fp32.
    - scores computed transposed sT[k, q] with the two heads of an m-tile
      row-packed into the 128-row array (tile_position via base partitions).
    - softmax denominator comes out of the PV matmul via a ones-column
      appended to V (planted once by a memset); normalization uses
      reciprocal_approx_fast + a tiny broadcast matmul, then one DVE
      multiply into the bf16 out^T staging tile.
    - the qkv / out projections are EMITTED INTERLEAVED with the attention
      blocks so the PE processes projection matmuls while the scalar engine
      (the attention bottleneck: exp) works through score tiles.
"""

import os
import sys

for _p in ("/opt/trn_rl_repo", "/root/.axon_site/_ro/trn_rl_repo"):
    if os.path.isdir(_p) and _p not in sys.path:
        sys.path.insert(0, _p)

import numpy as np
import ml_dtypes
from contextlib import ExitStack

import concourse.bass as bass
import concourse.tile as tile
from concourse import bacc, mybir
from concourse.bass_utils import run_bass_kernel_spmd

B, S, D = 2, 2048, 1024
H, HD = 16, 64
NCORES = 8
LOCAL_H = 4           # heads per core
P = 128
KO = D // P           # 8 contraction sub-tiles for the projections
NQ = S // 512         # 4 q-tiles of 512
NKT = S // P          # 16 k-blocks of 128
F32 = mybir.dt.float32
F32R = mybir.dt.float32r
BF16 = mybir.dt.bfloat16
AF = mybir.ActivationFunctionType
SCALE = 1.0 / np.sqrt(HD)
BF = ml_dtypes.bfloat16


def _emit(tc, nc, xT, wqk, wv, wo, bqk, bv, maskd, ones64d, ones128d, y,
          has_qkv_bias):
    with ExitStack() as ctx:
        consts = ctx.enter_context(tc.tile_pool(name="consts", bufs=1))
        persis = ctx.enter_context(tc.tile_pool(name="persist", bufs=1))
        # PSUM: pp 2x1 bank (proj + rb), ps 2x2 banks (scores),
        # po 1x2 banks (PV accum pair) -> 8 banks total
        pp = ctx.enter_context(tc.tile_pool(name="pp", bufs=2, space="PSUM"))
        ps = ctx.enter_context(tc.tile_pool(name="ps", bufs=2, space="PSUM"))
        pop = ctx.enter_context(tc.tile_pool(name="po", bufs=1, space="PSUM"))
        work = ctx.enter_context(tc.tile_pool(name="work", bufs=4))
        ypool = ctx.enter_context(tc.tile_pool(name="yp", bufs=3))
        small = ctx.enter_context(tc.tile_pool(name="small", bufs=2))

        # ---- constants ----
        mask128 = consts.tile([P, P], BF16)
        nc.gpsimd.dma_start(mask128[:], maskd[:, :])

        if has_qkv_bias:
            bqk_sb = consts.tile([P, 4], F32)
            nc.gpsimd.dma_start(bqk_sb[:], bqk.rearrange("(m p) -> p m", p=P))
            bv_sb = consts.tile([1, 256], BF16)
            nc.gpsimd.dma_start(bv_sb[:], bv[None, :])
            ones128_sb = consts.tile([1, P], BF16)
            nc.gpsimd.dma_start(ones128_sb[:], ones128d[None, :])

        # ---- weight / activation input DMAs (fine-grained so the first
        #      projection group can start ~4us in) ----
        wqk_t, wv_t = [], []
        x_t = [[None] * NQ for _ in range(KO)]
        for ko in range(KO):
            w = consts.tile([P, 512], BF16, name=f"wqk{ko}")
            # split the critical first weights across two queues
            (nc.gpsimd if ko % 2 == 0 else nc.scalar).dma_start(
                w[:], wqk[ko * P:(ko + 1) * P, :])
            wqk_t.append(w)
            t = persis.tile([P, 512], BF16, name=f"x{ko}_0")
            nc.sync.dma_start(t[:], xT[ko * P:(ko + 1) * P, 0:512])
            x_t[ko][0] = t
        for ko in range(KO):
            w = consts.tile([P, 256], BF16, name=f"wv{ko}")
            (nc.gpsimd if ko % 2 == 0 else nc.scalar).dma_start(
                w[:], wv[ko * P:(ko + 1) * P, :])
            wv_t.append(w)
            t = persis.tile([P, 512], BF16, name=f"x{ko}_1")
            nc.sync.dma_start(t[:], xT[ko * P:(ko + 1) * P, 512:1024])
            x_t[ko][1] = t
        wo_sb = consts.tile([P, 2, D], BF16)
        nc.gpsimd.dma_start(wo_sb[:], wo.rearrange("(ks p) n -> p ks n", p=P))
        for n in (2, 3):
            for ko in range(KO):
                t = persis.tile([P, 512], BF16, name=f"x{ko}_{n}")
                nc.sync.dma_start(t[:], xT[ko * P:(ko + 1) * P,
                                            n * 512:(n + 1) * 512])
                x_t[ko][n] = t

        # persistent activations
        qkT = persis.tile([P, 4, S], BF16)        # m 0,1: qT(h0..h3); 2,3: kT
        v_all = persis.tile([P, NKT, LOCAL_H, 65], BF16)
        outT = persis.tile([P, 2, S], BF16)       # attention out^T (bf16)

        # plant the softmax-denominator ones column of V once
        nc.vector.memset(v_all[:, :, :, 64:65], 1.0)

        def qk_group(m, n):
            """qkT[m][n-slice] = (wqk[:, m*128:+128]).T @ xT[:, n*512:+512]"""
            t = pp.tile([P, 512], F32, tag="p", name=f"qk{m}_{n}")
            for ko in range(KO):
                nc.tensor.matmul(
                    t[:],
                    wqk_t[ko][:, m * P:(m + 1) * P],
                    x_t[ko][n][:],
                    start=(ko == 0), stop=(ko == KO - 1),
                )
            dst = qkT[:, m, n * 512:(n + 1) * 512]
            if has_qkv_bias:
                nc.scalar.activation(dst, t[:], AF.Identity,
                                     bias=bqk_sb[:, m:m + 1])
            else:
                nc.any.tensor_copy(dst, t[:])

        def v_group(mt):
            """v_all[:, mt] = x[mt-block] @ wv  (natural layout)"""
            t = pp.tile([P, 512], F32, tag="p", name=f"vp{mt}")
            pv = t[:, 0:256]
            last = KO - 1 if not has_qkv_bias else None
            for ko in range(KO):
                nc.tensor.matmul(
                    pv,
                    x_t[ko][mt // 4][:, (mt % 4) * P:(mt % 4 + 1) * P],
                    wv_t[ko][:],
                    start=(ko == 0),
                    stop=(ko == KO - 1 and not has_qkv_bias),
                )
            if has_qkv_bias:
                nc.tensor.matmul(pv, ones128_sb[:1, :], bv_sb[:1, :],
                                 start=False, stop=True)
            nc.any.tensor_copy(
                v_all[:, mt, :, 0:64],
                pv.rearrange("p (h d) -> p h d", h=LOCAL_H),
            )

        def attn_block(jq, hp):
            po_t = pop.tile([65, 2, 512], F32, tag="po", name=f"po{jq}_{hp}")
            last_kt = 4 * jq + 3
            for kt in range(last_kt + 1):
                rel = kt - 4 * jq
                f0 = 128 * rel if rel > 0 else 0
                s_t = ps.tile([P, 2, 512], F32, tag="s")
                for i in range(2):
                    poff = 64 * i
                    nc.tensor.matmul(
                        s_t[:, i, f0:512],
                        qkT[poff:poff + 64, 2 + hp, kt * P:(kt + 1) * P],
                        qkT[poff:poff + 64, hp,
                            jq * 512 + f0:(jq + 1) * 512],
                        start=True, stop=True,
                    )
                et = work.tile([P, 2, 512], BF16, tag="et")
                nc.scalar.activation(et[:, :, f0:512], s_t[:, :, f0:512],
                                     AF.Exp, scale=float(SCALE))
                if rel >= 0:   # mask the 128-wide triangle at [f0, f0+128)
                    for i in range(2):
                        nc.vector.tensor_tensor(
                            et[:, i, f0:f0 + 128], et[:, i, f0:f0 + 128],
                            mask128[:], mybir.AluOpType.mult)
                for i in range(2):
                    lh = 2 * hp + i
                    nc.tensor.matmul(
                        po_t[:, i, f0:512],
                        v_all[:, kt, lh, :],
                        et[:, i, f0:512],
                        start=(kt == 0), stop=(kt == last_kt),
                    )
            # normalize: stage po to SBUF (frees the PSUM pair early), 1/den
            # via fast DVE reciprocal, partition-broadcast on the idle
            # GpSimd, one DVE multiply into bf16 outT
            st = small.tile([64, 2, 512], F32, tag="st")
            nc.vector.tensor_copy(st[:], po_t[0:64, :, :])
            den_t = small.tile([1, 2, 512], F32, tag="den")
            nc.scalar.activation(den_t[:], po_t[64:65, :, :], AF.Copy)
            # NOTE: reciprocal_approx_fast requires its input at partition 0
            rf = small.tile([1, 2, 512], F32, tag="rf")
            nc.vector.reciprocal_approx_fast(rf[:], den_t[:])
            rbb = small.tile([64, 2, 512], F32, tag="rbb")
            nc.gpsimd.partition_broadcast(rbb[:], rf[:], channels=64)
            for i in range(2):
                nc.vector.tensor_tensor(
                    outT[64 * i:64 * i + 64, hp, jq * 512:(jq + 1) * 512],
                    st[0:64, i, :], rbb[:, i, :], mybir.AluOpType.mult)

        def out_proj(jq):
            for mt in range(4 * jq, 4 * jq + 4):
                for n2 in range(2):
                    t = pp.tile([P, 512], F32, tag="p", name=f"op{mt}_{n2}")
                    for ks in range(2):
                        nc.tensor.matmul(
                            t[:],
                            outT[:, ks, mt * P:(mt + 1) * P],
                            wo_sb[:, ks, n2 * 512:(n2 + 1) * 512],
                            start=(ks == 0), stop=(ks == 1),
                        )
                    yt = ypool.tile([P, 512], F32, tag="y")
                    # alternate copy engine + DMA queue so the writeout
                    # never serializes on one engine/queue
                    idx = mt * 2 + n2
                    if idx % 2 == 0:
                        nc.vector.tensor_copy(yt[:], t[:])
                    else:
                        nc.scalar.activation(yt[:], t[:], AF.Copy)
                    (nc.gpsimd if idx % 2 == 0 else nc.sync).dma_start(
                        y[mt * P:(mt + 1) * P, n2 * 512:(n2 + 1) * 512],
                        yt[:])

        # ---- interleaved emission schedule ----
        # proj groups are emitted one attention block ahead of their use so
        # the PE always has projection work to fill exp-latency stalls.
        qk_group(2, 0); qk_group(0, 0)
        v_group(0); v_group(1); v_group(2); v_group(3)
        qk_group(3, 0); qk_group(1, 0)
        attn_block(0, 0)
        qk_group(2, 1); qk_group(0, 1)
        v_group(4); v_group(5); v_group(6); v_group(7)
        attn_block(0, 1)
        qk_group(3, 1); qk_group(1, 1)
        attn_block(1, 0)
        qk_group(2, 2); qk_group(0, 2)
        v_group(8); v_group(9); v_group(10); v_group(11)
        out_proj(0)
        attn_block(1, 1)
        qk_group(3, 2); qk_group(1, 2)
        attn_block(2, 0)
        qk_group(2, 3); qk_group(0, 3)
        v_group(12); v_group(13); v_group(14); v_group(15)
        out_proj(1)
        attn_block(2, 1)
        qk_group(3, 3); qk_group(1, 3)
        attn_block(3, 0)
        out_proj(2)
        attn_block(3, 1)
        out_proj(3)


def build_nc(has_qkv_bias):
    nc = bacc.Bacc("TRN2", target_bir_lowering=False, debug=False,
                   num_devices=NCORES)
    xT = nc.dram_tensor("xT", [D, S], BF16, kind="ExternalInput")
    wqk = nc.dram_tensor("wqk", [D, 512], BF16, kind="ExternalInput")
    wv = nc.dram_tensor("wv", [D, 256], BF16, kind="ExternalInput")
    wo = nc.dram_tensor("wo", [2 * P, D], BF16, kind="ExternalInput")
    bqk = nc.dram_tensor("bqk", [512], F32, kind="ExternalInput")
    bv = nc.dram_tensor("bv", [256], BF16, kind="ExternalInput")
    maskd = nc.dram_tensor("maskd", [P, P], BF16, kind="ExternalInput")
    ones64d = nc.dram_tensor("ones64d", [64], BF16, kind="ExternalInput")
    ones128d = nc.dram_tensor("ones128d", [P], BF16, kind="ExternalInput")
    y = nc.dram_tensor("y", [S, D], F32, kind="ExternalOutput")
    with tile.TileContext(nc) as tc:
        _emit(tc, nc, xT.ap(), wqk.ap(), wv.ap(), wo.ap(), bqk.ap(), bv.ap(),
              maskd.ap(), ones64d.ap(), ones128d.ap(), y.ap(), has_qkv_bias)
    nc.compile()
    return nc


_NC_CACHE = {}


def _get_nc(has_qkv_bias):
    key = bool(has_qkv_bias)
    if key not in _NC_CACHE:
        _NC_CACHE[key] = build_nc(key)
    return _NC_CACHE[key]


def make_in_maps(x, qkv_w, qkv_b, out_w):
    """Per-core host-side sharding. Core c: batch c//4, heads 4*(c%4)..+3."""
    in_maps = []
    xTs = [np.ascontiguousarray(x[b].T).astype(BF) for b in range(B)]
    # scores are stored transposed sT[k, q]: keep q >= k (upper triangle)
    mask = np.triu(np.ones((P, P), np.float32)).astype(BF)
    ones64 = np.ones(64, np.float32)
    ones128 = np.ones(P, np.float32).astype(BF)
    for c in range(NCORES):
        b = c // (NCORES // B)
        g = c % (NCORES // B)
        h0 = LOCAL_H * g
        cols = slice(h0 * HD, (h0 + LOCAL_H) * HD)
        wq = qkv_w[:, cols]
        wk = qkv_w[:, D:][:, cols]
        wv_ = qkv_w[:, 2 * D:][:, cols]
        bq = qkv_b[cols]
        bk = qkv_b[D:][cols]
        bv_ = qkv_b[2 * D:][cols]
        in_maps.append({
            "xT": xTs[b],
            "wqk": np.concatenate([wq, wk], axis=1).astype(BF),
            "wv": np.ascontiguousarray(wv_).astype(BF),
            "wo": np.ascontiguousarray(out_w[cols, :]).astype(BF),
            "bqk": np.ascontiguousarray(np.concatenate([bq, bk])),
            "bv": bv_.astype(BF),
            "maskd": mask,
            "ones64d": ones64.astype(BF),
            "ones128d": ones128,
        })
    return in_maps


def _ensure_ntff_hook():
    """Provide antenv.axon_hooks (missing in this image) so trace=True works."""
    try:
        from antenv.axon_hooks import get_axon_ntff_profile_hook  # noqa: F401
        return
    except ImportError:
        pass
    import types
    import antenv
    mod = types.ModuleType("antenv.axon_hooks")
    holder = {"hook": None}
    mod.set_axon_ntff_profile_hook = lambda h: holder.__setitem__("hook", h)
    mod.get_axon_ntff_profile_hook = lambda: holder["hook"]
    sys.modules["antenv.axon_hooks"] = mod
    antenv.axon_hooks = mod
    try:
        from trn_agent_boot.trn_boot import _ntff_profile_via_ctypes
        so = "/opt/axon/libaxon_pjrt.so"
        if os.path.exists(so):
            mod.set_axon_ntff_profile_hook(_ntff_profile_via_ctypes(so))
    except Exception:
        pass


def kernel(x, qkv_w, qkv_b, out_w, out_b, _trace=False):
    if _trace:
        _ensure_ntff_hook()
    x = np.asarray(x, dtype=np.float32)
    qkv_w = np.asarray(qkv_w, dtype=np.float32)
    qkv_b = np.asarray(qkv_b, dtype=np.float32)
    out_w = np.asarray(out_w, dtype=np.float32)
    out_b = np.asarray(out_b, dtype=np.float32)

    has_qkv_bias = bool(np.any(qkv_b))
    nc = _get_nc(has_qkv_bias)
    in_maps = make_in_maps(x, qkv_w, qkv_b, out_w)
    res = run_bass_kernel_spmd(nc, in_maps, core_ids=list(range(NCORES)),
                               trace=_trace)
    y = np.zeros((B, S, D), dtype=np.float32)
    for c in range(NCORES):
        y[c // (NCORES // B)] += res.results[c]["y"]
    y += out_b
    if _trace:
        kernel.last_results = res
    return y
